# revision 65
# baseline (speedup 1.0000x reference)
"""Distributed Bass kernel for nn_AttentionLayer (2-branch GAT-style layer).

Row-shard over 8 NeuronCores (512 rows each). All per-row tensors kept in
"transposed" layout on chip (k on SBUF partitions, own-row i on free) so the
masked softmax feeds the PE attention matmuls without transposes:

  e_b^T[k, i] = prelu(s1_b[i] + s2_b[k])          (ACT, or DVE tensor_scalar
                                                   + STT max(0.2z, z))
  p = exp(e)  (bf16) ;  pt = p * mask01           (multiplicative masking)
  [acc_b; den_b] = [Wh_b | 1]^T @ pt              (fused numerator+denominator)

adj2^T is computed on PE in fp8 DoubleRow (exact: adj is 0/1, psum f32) from a
REPLICATED full-adj input laid out host-side in DoubleRow tile order.

v2 schedule: the DR chain is the uninterrupted PE spine (16 groups x 2
column-tiles, cnt psum pool 4 banks deep = 2 groups of slack). All other work
is paced against it so no engine queue ever blocks the PE and the HAM
activity throttle stays released:
  - Wh tiles + s1 prologue are interleaved INTO the spine's tp passes
    (psum slot consumers get a full tp pass of latency cover).
  - branch-1 softmax+attention runs on its own early track (~3 kt/group),
    finishing by group 12 so its BN-stats AllReduce hides under the spine.
  - branch-2 drains one group behind the spine; its mask STTs are issued
    at the head of each iteration so cnt banks release promptly.
  - only branch-2's tiny stats AllReduce + BN affine remain after the spine;
    the sqrt ACT table is preloaded during the AllReduce wait.
"""

import sys
import numpy as np

for _p in ("/opt/trn_rl_repo", "/opt/trn_rl_repo/concourse"):
    if _p not in sys.path:
        sys.path.insert(0, _p)

import ml_dtypes

N = 4096
M_CORES = 8
R = N // M_CORES          # 512 rows per core
IN_F = 512
HALF = IN_F // 2          # 256
F = 64
P = 128                   # partitions
NT = N // P               # 32 column (k) tiles
G = 4                     # kt tiles per group (4-bank psum rotation keeps
                          # the DR chain at full rate)
NG = NT // G              # 8 groups
TP = 8                    # t-pair passes per group (16 t passes)
CB = G * P                # 512 columns per group
ALPHA = 0.2
EPS = 1e-5
INV_N = 1.0 / N

# per-group pacing tables (see header):
#   wh: Wh-tile production (interleaved into the spine's tp passes)
#   b1: branch-1 softmax/attention track
WH_QUOTA = [4, 4, 4, 4, 4, 4, 4, 4]
B1_QUOTA = [0, 0, 4, 5, 6, 6, 6, 5]
assert sum(WH_QUOTA) == NT and sum(B1_QUOTA) == NT

_CACHED = {}


def build_nc():
    from concourse import bacc, tile, mybir

    f32 = mybir.dt.float32
    bf16 = mybir.dt.bfloat16
    fp8 = mybir.dt.float8e4
    Alu = mybir.AluOpType
    Act = mybir.ActivationFunctionType
    DR = mybir.MatmulPerfMode.DoubleRow

    nc = bacc.Bacc("TRN2", target_bir_lowering=False, debug=False,
                   num_devices=M_CORES)

    hTs_p = nc.declare_dram_parameter("hTs", [P, 4, R], f32, isOutput=False)
    hTfb_p = nc.declare_dram_parameter("hTfb", [P, 4, N], bf16, isOutput=False)
    adjT_p = nc.declare_dram_parameter("adjT", [P, 16, 2, R], fp8,
                                       isOutput=False)
    adjDR_p = nc.declare_dram_parameter("adjDR", [NG, TP, P, 2, 2, CB], fp8,
                                        isOutput=False)
    dinv_p = nc.declare_dram_parameter("dinv", [P, NT, R], fp8, isOutput=False)
    Wsb_p = nc.declare_dram_parameter("Wsb", [P, 2, 2 * F], f32,
                                      isOutput=False)
    a1c_p = nc.declare_dram_parameter("a1c", [P, 1], f32, isOutput=False)
    a2bc_p = nc.declare_dram_parameter("a2bc", [P, F], f32, isOutput=False)
    gb4_p = nc.declare_dram_parameter("gb4", [F, 4], f32, isOutput=False)
    out_p = nc.declare_dram_parameter("out", [P, R], f32, isOutput=True)

    RG = [list(range(M_CORES))]

    with tile.TileContext(nc) as tc:
        with (
            tc.tile_pool(name="sb", bufs=1) as sb,
            tc.tile_pool(name="af", bufs=12) as afp,
            tc.tile_pool(name="ep", bufs=8) as epool,
            tc.tile_pool(name="pp", bufs=8) as ppool,
            tc.tile_pool(name="mp", bufs=4) as mpool,
            tc.tile_pool(name="ptp", bufs=6) as ptpool,
            tc.tile_pool(name="scp", bufs=4) as scpool,
            tc.tile_pool(name="pacc", bufs=1, space="PSUM") as pacc,
            tc.tile_pool(name="pcnt", bufs=5, space="PSUM") as pcnt,
            tc.tile_pool(name="ptmp", bufs=1, space="PSUM") as ptmp,
            tc.tile_pool(name="dram", bufs=1, space="DRAM") as dram,
        ):
            # ---- dummy collective at t=0: absorbs the one-time cc-stream
            # barrier so the real stats AllReduces are cheap.
            dumb_in = dram.tile([2, 1], f32)
            dumb_sb = sb.tile([2, 1], f32)
            nc.gpsimd.memset(dumb_sb[:], 0.0)
            nc.gpsimd.dma_start(dumb_in[:], dumb_sb[:])
            dumb_out = dram.tile([2, 1], f32, addr_space="Shared")
            nc.gpsimd.collective_compute(
                "AllReduce", Alu.add, replica_groups=RG,
                ins=[dumb_in[:].opt()], outs=[dumb_out[:].opt()])

            # ---- persistent loads ----
            # Spread the input streams over three rings by need-time so no
            # ring starves the spine: scalar carries adjT's lower half plus
            # the small prologue tensors, the vector ring carries adjT's
            # upper half, gpsimd carries hTs and the first hTfb chunks.
            # dinv and the later hTfb chunks are issued inside the loop.
            adjT = sb.tile([P, 16, 2, R], fp8)
            nc.scalar.dma_start(adjT[:, 0:2, :, :], adjT_p[:, 0:2, :, :])
            Wsb = sb.tile([P, 2, 2 * F], f32)
            nc.scalar.dma_start(Wsb[:], Wsb_p[:])
            a1c = sb.tile([P, 1], f32)
            nc.scalar.dma_start(a1c[:], a1c_p[:])
            a2bc = sb.tile([P, F], f32)
            nc.scalar.dma_start(a2bc[:], a2bc_p[:])
            nc.scalar.dma_start(adjT[:, 2:4, :, :], adjT_p[:, 2:4, :, :])
            nc.scalar.dma_start(adjT[:, 4:8, :, :], adjT_p[:, 4:8, :, :])
            nc.scalar.dma_start(adjT[:, 12:16, :, :], adjT_p[:, 12:16, :, :])
            dinv = sb.tile([P, NT, R], fp8)
            nc.scalar.dma_start(dinv[:, 0:4, :], dinv_p[:, 0:4, :])
            gb4 = sb.tile([F, 4], f32)
            nc.scalar.dma_start(gb4[:], gb4_p[:])

            # gpsimd ring: hTs (prologue), first hTfb chunk (Wh tiles for
            # group 0), adjT's third quarter, second hTfb chunk
            hTs = sb.tile([P, 4, R], f32)
            nc.gpsimd.dma_start(hTs[:], hTs_p[:])
            hTfb = sb.tile([P, 4, N], bf16)
            nc.gpsimd.dma_start(hTfb[:, :, 0:R], hTfb_p[:, :, 0:R])
            nc.gpsimd.dma_start(adjT[:, 8:12, :, :], adjT_p[:, 8:12, :, :])
            nc.gpsimd.dma_start(hTfb[:, :, R:2 * R], hTfb_p[:, :, R:2 * R])

            ones64 = sb.tile([P, F], f32)
            nc.vector.memset(ones64[:], 1.0)
            ones1 = sb.tile([1, P], f32)
            nc.vector.memset(ones1[:], 1.0)
            # bf16 copy of W for the natural-layout Wh matmuls (bf16 lhsT)
            Wsbb = sb.tile([P, 2, 2 * F], bf16)
            nc.vector.tensor_copy(Wsbb[:], Wsb[:])

            # ---- PE warm-up: dummy matmuls while the first DMAs stream.
            # The whole input ramp (~20us: adjT + af + hTs/hTfb contending
            # for ~100GB/s per DMA queue) must be covered HERE — filler
            # matmuls issued later sit behind the input-starved DR matmuls
            # in the in-order PE queue and never run. 512-wide matmuls on
            # memset data give ~213ns each with LDWEIGHTS hidden.
            onesb16 = sb.tile([P, F], bf16)
            nc.vector.memset(onesb16[:], 1.0)
            onesw = sb.tile([P, R], bf16)
            nc.vector.memset(onesw[:], 1.0)
            warm_ps = ptmp.tile([F, R], f32, tag="tmp", name="warm")
            for w in range(40):
                nc.tensor.matmul(warm_ps[:], onesb16[:], onesw[:],
                                 start=(w == 0), stop=(w == 39))

            # ---- psum accumulators: [0:64]=numerator, [64:65]=denominator
            acc = [pacc.tile([F + 1, R], f32, tag=f"acc{b}", name=f"acc{b}")
                   for b in range(2)]

            # ---- whf tiles (natural layout, bf16, branch-major) + s2 ----
            # whf[:, kt, b, 0:64]=Wh_b, whf[:, kt, b, 64]=1
            whf = sb.tile([P, NT, 2, F + 1], bf16)
            nc.vector.memset(whf[:, :, :, F:F + 1], 1.0)
            s2 = sb.tile([P, 2, NT], f32)
            s1bc = sb.tile([P, 2, R], f32)
            whT_sb = sb.tile([P, R], f32, name="whT_sb")
            s1_sb = [sb.tile([1, R], f32, tag=f"s1sb{b}", name=f"s1sb{b}")
                     for b in range(2)]

            def prologue_piece(step):
                # local Wh^T shard (f32) -> s1 -> s1bc (partition broadcast),
                # one piece per spine tp pass so psum-slot waits are covered.
                if step == 0:
                    whT_ps = ptmp.tile([P, R], f32, tag="tmp", name="whTps")
                    for b in range(2):
                        for t in range(2):
                            nc.tensor.matmul(
                                whT_ps[F * b:F * (b + 1), :],
                                Wsb[:, t, F * b:F * (b + 1)],
                                hTs[:, 2 * b + t, :],
                                start=(t == 0), stop=(t == 1),
                            )
                    nc.vector.tensor_copy(whT_sb[:], whT_ps[:])
                elif step in (1, 3):
                    b = 0 if step == 1 else 1
                    s1_ps = ptmp.tile([1, R], f32, tag="tmp", name=f"s1ps{b}")
                    nc.tensor.matmul(s1_ps[:], a1c[F * b:F * (b + 1), :],
                                     whT_sb[F * b:F * (b + 1), :],
                                     start=True, stop=True)
                    nc.vector.tensor_copy(s1_sb[b][:], s1_ps[:])
                elif step in (2, 4):
                    b = 0 if step == 2 else 1
                    bc_ps = ptmp.tile([P, R], f32, tag="tmp", name=f"s1bc{b}")
                    nc.tensor.matmul(bc_ps[:], ones1[:], s1_sb[b][:],
                                     start=True, stop=True)
                    nc.vector.tensor_copy(s1bc[:, b, :], bc_ps[:])

            def wh_tile(kt):
                # both branches accumulate into one single-bank psum tile;
                # ONE strided bf16 cast (vector) frees the slot, and the s2
                # reductions read the bf16 whf copy (off the slot path).
                whn = ptmp.tile([P, 2, F], f32, tag="tmp", name=f"whn{kt}")
                for b in range(2):
                    for t in range(2):
                        nc.tensor.matmul(
                            whn[:, b, :],
                            hTfb[:, 2 * b + t, P * kt:P * (kt + 1)],
                            Wsbb[:, t, F * b:F * (b + 1)],
                            start=(t == 0), stop=(t == 1),
                        )
                nc.vector.tensor_copy(whf[:, kt, :, 0:F], whn[:])
                for b in range(2):
                    scr = scpool.tile([P, F], bf16, tag="scr",
                                      name=f"s2s{kt}_{b}")
                    nc.vector.scalar_tensor_tensor(
                        scr[:], whf[:, kt, b, 0:F], 1.0, a2bc[:],
                        op0=Alu.mult, op1=Alu.mult,
                        accum_out=s2[:, b, kt:kt + 1])

            def att_mm(b, kt, pt):
                nc.tensor.matmul(acc[b][:], whf[:, kt, b, 0:F + 1],
                                 pt[:], start=(kt == 0), stop=(kt == NT - 1))

            def b1_do(kt):
                # branch-1 softmax chain; prelu alternates ACT / DVE and the
                # mask multiply runs on gpsimd so the vector queue stays short
                e = epool.tile([P, R], f32, tag="e")
                nc.scalar.activation(e[:], s1bc[:, 0, :], Act.Prelu,
                                     bias=s2[:, 0, kt:kt + 1], alpha=ALPHA)
                p = ppool.tile([P, R], bf16, tag="p")
                nc.scalar.activation(p[:], e[:], Act.Exp)
                pt = ptpool.tile([P, R], bf16, tag="pt")
                nc.vector.tensor_tensor(pt[:], p[:],
                                        adjT[:, kt // 2, kt % 2, :],
                                        op=Alu.mult)
                att_mm(0, kt, pt)

            def b2_exp(kt):
                e = epool.tile([P, R], f32, tag="e")
                nc.scalar.activation(e[:], s1bc[:, 1, :], Act.Prelu,
                                     bias=s2[:, 1, kt:kt + 1], alpha=ALPHA)
                p = ppool.tile([P, R], bf16, tag="p")
                nc.scalar.activation(p[:], e[:], Act.Exp)
                return p

            # ---- epilogue stats: normalize by the softmax denominator and
            # reduce sum / sum-of-squares for the BN batch stats. Both
            # branches share ONE AllReduce (each cc collective costs ~20us
            # of serial stream time regardless of size); branch 1's stats
            # are computed early (inside the spine) so only the DMA of its
            # half remains before the AllReduce can fire.
            stats_in = dram.tile([F, 2, 2], f32, name="stin")
            stats_out = dram.tile([F, 2, 2], f32, addr_space="Shared",
                                  name="stout")
            hp = []

            def epilogue_stats(b):
                # the custom-DVE reciprocal needs a partition-0-based AP, so
                # stage the denominator row on partition 0 first
                den = sb.tile([1, R], f32, tag=f"den{b}", name=f"den{b}")
                nc.vector.tensor_copy(den[:], acc[b][F:F + 1, :])
                rct = sb.tile([1, R], f32, tag=f"rct{b}", name=f"rct{b}")
                nc.vector.reciprocal_approx_fast(rct[:], den[:])
                rbc_ps = ptmp.tile([F, R], f32, tag="tmp", name=f"rbc{b}")
                nc.tensor.matmul(rbc_ps[:], ones64[0:1, :],
                                 rct[:], start=True, stop=True)
                rbc = sb.tile([F, R], f32, tag=f"rbc{b}", name=f"rbc{b}")
                nc.vector.tensor_copy(rbc[:], rbc_ps[:])
                hpb = sb.tile([F, R], f32, tag=f"hp{b}", name=f"hp{b}")
                nc.vector.tensor_mul(hpb[:], acc[b][0:F, :], rbc[:])
                hp.append(hpb)
                sx = sb.tile([F, 2], f32, tag=f"sx{b}", name=f"sx{b}")
                nc.vector.tensor_reduce(sx[:, 0:1], hpb[:],
                                        axis=mybir.AxisListType.X, op=Alu.add)
                scr = ppool.tile([P, R], bf16, tag="p", name=f"sq{b}")
                nc.scalar.activation(scr[0:F, :], hpb[:], Act.Square,
                                     accum_out=sx[:, 1:2])
                nc.gpsimd.dma_start(stats_in[:, b, :], sx[:])
                if b == 1:
                    nc.gpsimd.collective_compute(
                        "AllReduce", Alu.add, replica_groups=RG,
                        ins=[stats_in[:].opt()], outs=[stats_out[:].opt()])

            # ---- main loop: the DR spine with everything paced against it.
            # The last macro group is split into two 2-kt subgroups so the
            # final drain chain (mask STT -> multiply -> att) covers only 2
            # column tiles and the b2 stats AllReduce fires earlier.
            subgroups = [(g, 0, G) for g in range(NG - 2)]
            subgroups += [(NG - 2, 0, 2), (NG - 2, 2, 2),
                          (NG - 1, 0, 2), (NG - 1, 2, 2)]
            pend = None            # ([kts], cnts, [p2 tiles])
            wh_done = 0
            b1_done = 0

            for g, j0, nj in subgroups:
                # drain the previous subgroup: the mask STTs + multiplies are
                # issued at the head of the iteration (releasing the cnt psum
                # banks promptly), but the b2 att matmuls are staggered into
                # this subgroup's spine (tp 2+) so the in-order PE queue
                # never waits on the ~2us mask chain.
                pend_atts = []
                if pend is not None:
                    ktd, cd, p2d = pend
                    for j, kt in enumerate(ktd):
                        m = mpool.tile([P, R], bf16, tag="m")
                        nc.vector.scalar_tensor_tensor(
                            m[:], cd[j][:], 1.0, dinv[:, kt, :],
                            op0=Alu.min, op1=Alu.mult)
                        pt = ptpool.tile([P, R], bf16, tag="pt")
                        eng = nc.gpsimd if j % 2 == 0 else nc.vector
                        eng.tensor_tensor(pt[:], p2d[j][:], m[:],
                                          op=Alu.mult)
                        pend_atts.append((kt, pt))

                kts = [G * g + j0 + j for j in range(nj)]
                cnts = [pcnt.tile([P, R], f32, tag="cnt",
                                  name=f"cnt{g}_{j0 + j}")
                        for j in range(nj)]
                wh_g = WH_QUOTA[g] if j0 == 0 else 0
                wh_per_tp = [0] * TP
                for i in range(wh_g):
                    wh_per_tp[(i * TP) // wh_g] += 1
                if g < 6 and j0 == 0:
                    ch = g + 2
                    nc.gpsimd.dma_start(hTfb[:, :, R * ch:R * (ch + 1)],
                                        hTfb_p[:, :, R * ch:R * (ch + 1)])
                for tp in range(TP):
                    af = afp.tile([P, 2, 2, P * nj], fp8, tag="af")
                    nc.sync.dma_start(
                        af[:], adjDR_p[g, tp, :, :, :,
                                       P * j0:P * (j0 + nj)])
                    for j in range(nj):
                        for dt in range(2):
                            nc.tensor.matmul(
                                cnts[j][:],
                                af[:, dt, :, P * j:P * (j + 1)],
                                adjT[:, 2 * tp + dt, :, :],
                                perf_mode=DR,
                                start=(tp == 0 and dt == 0),
                                stop=(tp == TP - 1 and dt == 1),
                            )
                    if 2 <= tp < 2 + len(pend_atts):
                        att_mm(1, *pend_atts[tp - 2])
                    if g == 0 and 1 <= tp <= 5:
                        prologue_piece(tp - 1)
                    for _ in range(wh_per_tp[tp]):
                        wh_tile(wh_done)
                        wh_done += 1
                # later dinv chunks ride the scalar ring inside the loop so
                # the early-phase DMA stays under the HBM budget
                if g < NG - 1 and j0 == 0:
                    nc.scalar.dma_start(dinv[:, 4 * (g + 1):4 * (g + 2), :],
                                        dinv_p[:, 4 * (g + 1):4 * (g + 2), :])
                if g == NG - 1 and j0 == 0:
                    # fillers cover the last branch-1 chains' latency
                    warm_b = ptmp.tile([F, R], f32, tag="tmp", name="warmb")
                    for w in range(10):
                        nc.tensor.matmul(warm_b[:], onesb16[:], onesw[:],
                                         start=(w == 0), stop=(w == 9))

                # branch-1 early track
                if j0 == 0:
                    for _ in range(B1_QUOTA[g]):
                        b1_do(b1_done)
                        b1_done += 1
                    if b1_done == NT:
                        b1_done += 1     # fire once
                        epilogue_stats(0)

                # branch-2 exp production for this subgroup (consumed at the
                # next iteration's drain)
                p2s = [b2_exp(kt) for kt in kts]
                pend = (kts, cnts, p2s)

            # ---- tail: drain the last subgroup, b2 stats, BN, store ----
            ktd, cd, p2d = pend
            pts_tail = []
            for j, kt in enumerate(ktd):
                m = mpool.tile([P, R], bf16, tag="m")
                nc.vector.scalar_tensor_tensor(
                    m[:], cd[j][:], 1.0, dinv[:, kt, :],
                    op0=Alu.min, op1=Alu.mult)
                pt = ptpool.tile([P, R], bf16, tag="pt")
                eng = nc.gpsimd if j == 0 else nc.vector
                eng.tensor_tensor(pt[:], p2d[j][:], m[:], op=Alu.mult)
                pts_tail.append((kt, pt))
            # fillers absorb the tail mask-chain latency so the remaining
            # matmuls run at the released HAM rate instead of re-throttling
            warm_t = ptmp.tile([F, R], f32, tag="tmp", name="warmt")
            for w in range(12):
                nc.tensor.matmul(warm_t[:], onesb16[:], onesw[:],
                                 start=(w == 0), stop=(w == 11))
            for kt, pt in pts_tail:
                att_mm(1, kt, pt)
            epilogue_stats(1)
            # preload the sqrt ACT table while the AllReduce ring runs
            sq_warm = sb.tile([1, 2], f32, name="sqwarm")
            nc.scalar.activation(sq_warm[:], ones1[0:1, 0:2], Act.Sqrt)

            # BN coefficient math for both branches at once on [F, 2] tiles
            # (gb4 is [F, (b, gamma/beta)] in memory).
            gst = sb.tile([F, 2, 2], f32)
            nc.sync.dma_start(gst[:], stats_out[:])
            mex = sb.tile([F, 2, 2], f32)
            nc.vector.tensor_scalar_mul(mex[:], gst[:], INV_N)
            mean = mex[:, :, 0]
            var = sb.tile([F, 2], f32)
            nc.vector.scalar_tensor_tensor(var[:], mean, -1.0, mean,
                                           op0=Alu.mult, op1=Alu.mult)
            nc.vector.tensor_add(var[:], var[:], mex[:, :, 1])
            nc.vector.tensor_scalar_add(var[:], var[:], EPS)
            std = sb.tile([F, 2], f32)
            nc.scalar.activation(std[:], var[:], Act.Sqrt)
            rstd = sb.tile([F, 2], f32)
            nc.vector.reciprocal(rstd[:], std[:])
            gb4v = gb4[:].rearrange("f (b t) -> f b t", b=2)
            scale = sb.tile([F, 2], f32)
            nc.vector.tensor_mul(scale[:], gb4v[:, :, 0], rstd[:])
            nbias = sb.tile([F, 2], f32)
            nc.vector.scalar_tensor_tensor(nbias[:], mean, -1.0,
                                           scale[:], op0=Alu.mult,
                                           op1=Alu.mult)
            nc.vector.tensor_add(nbias[:], nbias[:], gb4v[:, :, 1])
            for b in range(2):
                # BN affine + final leakyrelu fused in one activation
                fin = sb.tile([F, R], f32, tag=f"fin{b}", name=f"fin{b}")
                nc.scalar.activation(fin[:], hp[b][:], Act.Prelu,
                                     bias=nbias[:, b:b + 1],
                                     scale=scale[:, b:b + 1],
                                     alpha=ALPHA)
                nc.gpsimd.dma_start(out_p[F * b:F * (b + 1), :], fin[:])

    nc.compile()
    return nc


def _get_nc():
    if "nc" not in _CACHED:
        _CACHED["nc"] = build_nc()
    return _CACHED["nc"]


def make_in_maps(h, adj, W1, W2, a, gamma, beta):
    fp8 = ml_dtypes.float8_e4m3fn
    bf16 = ml_dtypes.bfloat16
    h = np.asarray(h, dtype=np.float32)
    adj = np.asarray(adj, dtype=np.float32)
    W1 = np.asarray(W1, np.float32)
    W2 = np.asarray(W2, np.float32)
    a_flat = np.asarray(a, np.float32).reshape(2 * F)
    gamma = np.asarray(gamma, np.float32)
    beta = np.asarray(beta, np.float32)

    adj8 = adj.astype(fp8)
    # adjDR[g, tp, p, dt, s, u] = adj[256*(2tp+dt) + 128*s + p, 256*g + u]
    t1 = adj8.reshape(TP, 2, 2, P, NG, CB)         # [tp, dt, s, p, g, u]
    adjDR = np.ascontiguousarray(t1.transpose(4, 0, 3, 1, 2, 5))

    hT = h.T                                        # [IN_F, N]
    hTfb = np.ascontiguousarray(
        hT.astype(bf16).reshape(4, P, N).transpose(1, 0, 2))

    Wsb = np.ascontiguousarray(
        np.concatenate([W1, W2], axis=1).reshape(2, P, 2 * F)
        .transpose(1, 0, 2))
    a1c = np.ascontiguousarray(
        np.concatenate([a_flat[:F], a_flat[:F]]).reshape(P, 1))
    a2bc = np.ascontiguousarray(
        np.broadcast_to(a_flat[F:], (P, F)))
    gb4 = np.ascontiguousarray(
        np.stack([gamma[:F], beta[:F], gamma[F:], beta[F:]], axis=1))

    in_maps = []
    for c in range(M_CORES):
        r0 = c * R
        # adjT[p, t, s, i] = adj[r0+i, 256t+128s+p]
        ash = adj8[r0:r0 + R, :].T                  # [N(t_glob), R(i)]
        adjT = np.ascontiguousarray(
            ash.reshape(16, 2, P, R).transpose(2, 0, 1, 3))
        # dinv[p, kt, i] = 0 where 128*kt + p == r0 + i
        dinv = np.ones((P, NT, R), dtype=fp8)
        ii = np.arange(R)
        kk = r0 + ii
        dinv[kk % P, kk // P, ii] = 0
        hTs = np.ascontiguousarray(
            hT[:, r0:r0 + R].reshape(4, P, R).transpose(1, 0, 2))
        in_maps.append({
            "hTs": hTs,
            "hTfb": hTfb,
            "adjT": adjT,
            "adjDR": adjDR,
            "dinv": dinv,
            "Wsb": Wsb,
            "a1c": a1c,
            "a2bc": a2bc,
            "gb4": gb4,
        })
    return in_maps


def kernel(h, adj, W1, W2, a, gamma, beta):
    from concourse.bass_utils import run_bass_kernel_spmd

    in_maps = make_in_maps(h, adj, W1, W2, a, gamma, beta)
    nc = _get_nc()
    res = run_bass_kernel_spmd(nc, in_maps, core_ids=list(range(M_CORES)))
    out = np.empty((N, 2 * F), dtype=np.float32)
    for c in range(M_CORES):
        out[c * R:(c + 1) * R, :] = np.asarray(res.results[c]["out"]).T
    return out
